# revision 16
# baseline (speedup 1.0000x reference)
"""DiscoDownBlock Trainium2 kernel.

Sharding: 8 cores = B(2) x W-quarters(4). Host pre-rotates x along W per core
so every core runs an identical program (SPMD) with identical baked gather
offsets; only the data differs. Each core:
  P1: full-batch GroupNorm+MLP recompute (bf16), residual; writes its
      W-quarter of `skip` in fp32 and keeps full skip (bf16) in SBUF.
  P2: DISCO sparse gather (static SBUF->SBUF DMAs; stride-2 circular slices
      of skip rows) into (h,n)-partition tiles; basis reduce on TensorE via
      per-w stationary loads (transpose for free); channel contraction
      against prepacked weights.
  P3: GroupNorm2 (partial stats + tiny AllReduce) + GELU -> x_down quarter.
"""

import math
import numpy as np
import ml_dtypes
from contextlib import ExitStack

import concourse.bass as bass
import concourse.mybir as mybir
import concourse.tile as tile
import concourse.tile_sem_assignment as _tsa
from concourse.bass_utils import run_bass_kernel_spmd

# Cap the number of DMA completion-sem lanes Tile distributes DMAs over.
# With the default 8 HW + 8 SW lanes, instructions that depend on many DMAs
# (pool-phase transitions, gather-tile consumers) accumulate more sync-wait
# slots than the TPB instruction encoding allows ("Too many sync wait
# commands" in walrus codegen). 3+2 lanes keeps every consumer <= 8 waits.
_tsa.NUM_HWDGE_SEMS = 3
_tsa.NUM_SWDGE_GLOBAL_SEMS = 2

BF = mybir.dt.bfloat16
F32 = mybir.dt.float32
AF = mybir.ActivationFunctionType
ALU = mybir.AluOpType

HIN, WIN, HOUT, WOUT = 181, 360, 91, 180
CIN, COUT, K, NNZ, B = 128, 256, 7, 16, 2
NW = 4                      # W-quarters
WQ = WIN // NW              # 90 input cols written per core
WOQ = WOUT // NW            # 45 output cols per core
NPIX = HIN * WIN            # 65160
HT = 12                     # h tiles of 8 slots -> 96 (91 real + 5 pad)
HHALF = 48                  # h slots per y_T half
NLOC = HOUT * WOQ           # 4095 pixels per channel in this quarter
NGLOB = float(HOUT * WOUT * 32)   # gn2 group element count (524160)
EPS = 1e-5

_CACHE = {}


def _legalize_waits(nc):
    """The TPB instruction encoding has a single sync-wait slot; walrus
    refuses instructions carrying more ("Too many sync wait commands").
    Tile emits multi-wait sync_info freely, so split the extras into
    standalone EventSemaphore instructions on the same engine sequencer,
    placed immediately before the instruction (raw-bass wait_ge style)."""
    import json as _json
    d = _json.loads(mybir.module_to_json_string(nc.m))
    ctr = 0
    for f in d["functions"]:
        bbs = f.get("basic_blocks") or f.get("blocks") or []
        for bb in bbs:
            out = []
            for inst in bb["instructions"]:
                si = inst.get("sync_info")
                if si:
                    w = si.get("on_wait") or []
                    if len(w) > 1:
                        eng = inst.get("engine")
                        for extra in w[:-1]:
                            ctr += 1
                            out.append({
                                "opcode": "EventSemaphore",
                                "name": f"I-waitx-{ctr}",
                                "engine": eng,
                                "ins": [],
                                "outs": [],
                                "sync_info": {"on_wait": [extra],
                                              "on_update": []},
                            })
                        si["on_wait"] = w[-1:]
                out.append(inst)
            bb["instructions"] = out
    nc.m = mybir.module_from_json_bytes(
        _json.dumps(d).encode())
    return ctr


def _gather_pieces(psi_hi, psi_wi):
    """Static (w0, ln, src_off) pieces per (h-slot, n) for the circular
    gather from the de-interleaved skip copy (dei free layout =
    (hi, parity, 180)), in rotated coordinates (identical on every core)."""
    pieces = []
    for hh in range(HT * 8):
        row = []
        for n in range(NNZ):
            if hh < HOUT:
                hi = int(psi_hi[hh, n]) % HIN
                wi = int(psi_wi[hh, n]) % WIN
            else:
                hi, wi = 0, 0
            par = wi & 1
            st = wi >> 1
            base = hi * WIN + par * (WIN // 2)
            if st + WOQ <= WIN // 2:
                row.append([(0, WOQ, base + st)])
            else:
                l1 = WIN // 2 - st
                row.append([(0, l1, base + st), (l1, WOQ - l1, base)])
        pieces.append(row)
    return pieces


def _build_program(psi_hi, psi_wi):
    nc = bass.Bass()

    xr_bf = nc.declare_dram_parameter("xr_bf", [CIN, NPIX], BF, isOutput=False)
    xr_f32q = nc.declare_dram_parameter("xr_f32q", [CIN, HIN, WQ], F32, isOutput=False)
    w1T_d = nc.declare_dram_parameter("w1T", [CIN, COUT], BF, isOutput=False)
    w2T_d = nc.declare_dram_parameter("w2T", [CIN, 2, CIN], BF, isOutput=False)
    wTk_d = nc.declare_dram_parameter("wTk", [CIN, K, COUT], BF, isOutput=False)
    psiBD_d = nc.declare_dram_parameter("psiBD", [CIN, HT, 56], BF, isOutput=False)
    c128_d = nc.declare_dram_parameter("c128", [CIN, 3], F32, isOutput=False)
    c256_d = nc.declare_dram_parameter("c256", [CIN, 2, 4], F32, isOutput=False)
    g1_d = nc.declare_dram_parameter("g1", [CIN, 8], F32, isOutput=False)
    g1t_d = nc.declare_dram_parameter("g1t", [8, CIN], F32, isOutput=False)
    g2_d = nc.declare_dram_parameter("g2", [CIN, 4], F32, isOutput=False)
    g2t_d = nc.declare_dram_parameter("g2t", [4, CIN], F32, isOutput=False)

    skip_q = nc.declare_dram_parameter("skip_q", [CIN, HIN, WQ], F32, isOutput=True)
    xdown_q = nc.declare_dram_parameter("xdown_q", [2, CIN, HOUT, WOQ], F32, isOutput=True)

    out_sc = nc.dram_tensor("out_sc", [2, CIN, HT * 8, WOQ], F32)

    pieces = _gather_pieces(psi_hi, psi_wi)

    with tile.TileContext(nc) as tc, ExitStack() as ctx:
        wpool = ctx.enter_context(tc.tile_pool(name="weights", bufs=1))
        deipool = ctx.enter_context(tc.tile_pool(name="dei", bufs=1))
        spool = ctx.enter_context(tc.tile_pool(name="small", bufs=1))
        drampool = ctx.enter_context(tc.tile_pool(name="dram", bufs=1, space="DRAM"))

        w1T = wpool.tile([CIN, COUT], BF)
        nc.sync.dma_start(w1T[:], w1T_d[:])
        w2T = wpool.tile([CIN, 2, CIN], BF)
        nc.sync.dma_start(w2T[:], w2T_d[:])
        wTk = wpool.tile([CIN, K, COUT], BF)
        nc.sync.dma_start(wTk[:], wTk_d[:])
        psiBD = wpool.tile([CIN, HT, 56], BF)
        nc.sync.dma_start(psiBD[:], psiBD_d[:])
        c128 = wpool.tile([CIN, 3], F32)
        nc.sync.dma_start(c128[:], c128_d[:])
        c256 = wpool.tile([CIN, 2, 4], F32)
        nc.sync.dma_start(c256[:], c256_d[:])
        g1m = wpool.tile([CIN, 8], F32)
        nc.sync.dma_start(g1m[:], g1_d[:])
        g1tm = wpool.tile([8, CIN], F32)
        nc.sync.dma_start(g1tm[:], g1t_d[:])
        g2m = wpool.tile([CIN, 4], F32)
        nc.sync.dma_start(g2m[:], g2_d[:])
        g2tm = wpool.tile([4, CIN], F32)
        nc.sync.dma_start(g2tm[:], g2t_d[:])

        dei = deipool.tile([CIN, NPIX], BF)

        # ---------------- P1a: gn1 stats over full batch image ----------------
        NCH1 = (NPIX + 511) // 512   # 128
        stats1 = spool.tile([CIN, NCH1, 6], F32)
        sc1 = spool.tile([CIN, 2], F32)     # [:,0]=scale1, [:,1]=bias1
        with tc.tile_pool(name="p1a", bufs=4) as p1a, \
             tc.tile_pool(name="p1apsum", bufs=1, space="PSUM") as pp1a:
            for i in range(NCH1):
                off = i * 512
                ln = min(512, NPIX - off)
                t = p1a.tile([CIN, 512], BF)
                nc.sync.dma_start(t[:, :ln], xr_bf[:, off:off + ln])
                nc.vector.bn_stats(stats1[:, i, :], t[:, :ln])
            ch1 = spool.tile([CIN, 2], F32)
            nc.vector.bn_aggr(
                ch1[:], stats1[:].rearrange("p a b -> p (a b)").rearrange(
                    "p (n t) -> p n t", t=3))
            me1 = spool.tile([CIN, 2], F32)     # (mean, E[x^2]) per channel
            nc.vector.tensor_copy(me1[:, 0:1], ch1[:, 0:1])
            tmp1 = spool.tile([CIN, 1], F32)
            nc.vector.tensor_mul(tmp1[:], ch1[:, 0:1], ch1[:, 0:1])
            nc.vector.tensor_add(me1[:, 1:2], ch1[:, 1:2], tmp1[:])
            gp = pp1a.tile([8, 2], F32)
            nc.tensor.matmul(gp[:], g1m[:], me1[:])
            gstat = spool.tile([8, 2], F32)     # (mu_g, E2_g)
            nc.scalar.mul(gstat[:], gp[:], 1.0 / 16.0)
            gv = spool.tile([8, 1], F32)
            nc.vector.tensor_mul(gv[:], gstat[:, 0:1], gstat[:, 0:1])
            nc.vector.tensor_sub(gv[:], gstat[:, 1:2], gv[:])
            nc.vector.tensor_scalar_add(gv[:], gv[:], EPS)
            gr = spool.tile([8, 2], F32)        # [:,0]=mu, [:,1]=rstd
            nc.vector.tensor_copy(gr[:, 0:1], gstat[:, 0:1])
            nc.vector.reciprocal(gr[:, 1:2], gv[:])
            nc.scalar.sqrt(gr[:, 1:2], gr[:, 1:2])
            bc = pp1a.tile([CIN, 2], F32)
            nc.tensor.matmul(bc[:], g1tm[:], gr[:])
            nc.vector.tensor_mul(sc1[:, 0:1], c128[:, 0:1], bc[:, 1:2])
            t1b = spool.tile([CIN, 1], F32)
            nc.vector.tensor_mul(t1b[:], bc[:, 0:1], sc1[:, 0:1])
            nc.vector.tensor_sub(sc1[:, 1:2], c128[:, 1:2], t1b[:])

        # ---------------- P1b: MLP + residual, 2-row chunks ----------------
        b2ap = c128[:, 2:3]
        with tc.tile_pool(name="p1b", bufs=3) as p1b, \
             tc.tile_pool(name="p1bq", bufs=3) as p1bq, \
             tc.tile_pool(name="m1", bufs=1, space="PSUM") as m1p, \
             tc.tile_pool(name="m2", bufs=2, space="PSUM") as m2p:
            nch = (HIN + 1) // 2
            for j in range(nch):
                r0 = 2 * j
                nr = min(2, HIN - r0)
                px = nr * WIN
                xb = p1b.tile([CIN, 2, WIN], BF, tag="xb")
                nc.sync.dma_start(
                    xb[:, :nr, :].rearrange("p a b -> p (a b)"),
                    xr_bf[:, r0 * WIN: r0 * WIN + px])
                xbf = xb[:].rearrange("p a b -> p (a b)")
                xn = p1b.tile([CIN, 2 * WIN], BF, tag="xn")
                nc.scalar.activation(xn[:, :px], xbf[:, :px], AF.Identity,
                                     bias=sc1[:, 1:2], scale=sc1[:, 0:1])
                m1 = m1p.tile([CIN, 2, 2, 512], F32)
                for oh in range(2):
                    for s in range(nr):
                        nc.tensor.matmul(
                            m1[:, oh, s, :WIN],
                            w1T[:, oh * 128:(oh + 1) * 128],
                            xn[:, s * WIN:(s + 1) * WIN])
                g1b = p1b.tile([CIN, 2, 2, WIN], BF, tag="g1b")
                for oh in range(2):
                    for s in range(nr):
                        nc.scalar.activation(
                            g1b[:, oh, s, :], m1[:, oh, s, :WIN],
                            AF.Gelu_apprx_tanh,
                            bias=c256[:, oh, 0:1], scale=1.0)
                m2 = m2p.tile([CIN, 2, 512], F32)
                for s in range(nr):
                    for kk in range(2):
                        nc.tensor.matmul(
                            m2[:, s, :WIN], w2T[:, kk, :], g1b[:, kk, s, :],
                            start=(kk == 0), stop=(kk == 1))
                deiv = dei[:].rearrange("p (h q) -> p h q", q=WIN)
                for par in range(2):
                    nc.vector.scalar_tensor_tensor(
                        deiv[:, r0:r0 + nr, par * 180:(par + 1) * 180],
                        m2[:, :nr, par:WIN:2], b2ap,
                        xb[:, :nr, par:WIN:2],
                        op0=ALU.add, op1=ALU.add)
                xq = p1bq.tile([CIN, 2, WQ], F32, tag="xq")
                nc.sync.dma_start(xq[:, :nr, :], xr_f32q[:, r0:r0 + nr, :])
                sq = p1bq.tile([CIN, 2, WQ], F32, tag="sq")
                nc.vector.scalar_tensor_tensor(
                    sq[:, :nr, :], m2[:, :nr, :WQ], b2ap, xq[:, :nr, :],
                    op0=ALU.add, op1=ALU.add)
                nc.scalar.dma_start(skip_q[:, r0:r0 + nr, :], sq[:, :nr, :])

        # ---------------- P2: DISCO gather + basis reduce + contraction -------
        st2 = spool.tile([CIN, 2, 16, 6], F32)
        dma_engines = [nc.sync]
        HQRT = 24
        with tc.tile_pool(name="yT", bufs=1) as ypool, \
             tc.tile_pool(name="gat", bufs=2) as gpool, \
             tc.tile_pool(name="outst", bufs=3) as opool, \
             tc.tile_pool(name="apsum", bufs=2, space="PSUM") as apool, \
             tc.tile_pool(name="bpsum", bufs=4, space="PSUM") as bpool:
            eng_i = 0
            for q in range(4):
                yT = ypool.tile([CIN, K, HQRT, WOQ], BF)
                for t3 in range(3):
                    t = q * 3 + t3
                    g = gpool.tile([128, CIN, WOQ], BF)
                    for hl in range(8):
                        for n in range(NNZ):
                            p0 = hl * 16 + n
                            for (w0, ln, s0) in pieces[t * 8 + hl][n]:
                                eng = dma_engines[eng_i % len(dma_engines)]
                                eng_i += 1
                                eng.dma_start(
                                    g[p0:p0 + 1, :, w0:w0 + ln],
                                    dei[:, s0: s0 + ln])
                    ap_t = None
                    for w in range(WOQ):
                        sl = w % 4
                        if sl == 0:
                            ap_t = apool.tile([CIN, 4, 56], F32)
                        nc.tensor.matmul(ap_t[:, sl, :], g[:, :, w],
                                         psiBD[:, t, :])
                        if sl == 3 or w == WOQ - 1:
                            wb = w - sl
                            nc.vector.tensor_copy(
                                yT[:, :, t3 * 8: t3 * 8 + 8, wb: w + 1]
                                .rearrange("p k h w -> p w h k"),
                                ap_t[:, : sl + 1, :]
                                .rearrange("p w (h k) -> p w h k", k=K))
                for pr in range(2):
                    bts = {}
                    for oh in range(2):
                        for kk in range(K):
                            for s2 in range(2):
                                sup = pr * 2 + s2
                                if (oh, s2) not in bts:
                                    bts[(oh, s2)] = bpool.tile(
                                        [CIN, 6 * WOQ], F32, name="bts",
                                        tag="bts")
                                nc.tensor.matmul(
                                    bts[(oh, s2)][:],
                                    wTk[:, kk, oh * 128:(oh + 1) * 128],
                                    yT[:, kk, sup * 6:(sup + 1) * 6, :]
                                    .rearrange("p h w -> p (h w)"),
                                    start=(kk == 0), stop=(kk == K - 1))
                    for oh in range(2):
                        for s2 in range(2):
                            sup = pr * 2 + s2
                            habs = q * HQRT + sup * 6
                            nreal = max(0, min(6, HOUT - habs))
                            ot = opool.tile([CIN, 6 * WOQ], F32)
                            nc.vector.tensor_scalar(
                                ot[:], bts[(oh, s2)][:], c256[:, oh, 1:2],
                                None, op0=ALU.add)
                            if nreal > 0:
                                nc.vector.bn_stats(
                                    st2[:, oh, q * 4 + pr * 2 + s2, :],
                                    ot[:, : nreal * WOQ])
                            nc.sync.dma_start(
                                out_sc[oh, :, habs: habs + 6, :]
                                .rearrange("p a b -> p (a b)"),
                                ot[:])

            # gn2 stats: per-channel aggregate -> group sums -> AllReduce
            # layout [4 groups (partitions), (oh, 2)] to stay partition-aligned
            with tc.tile_pool(name="p2psum", bufs=1, space="PSUM") as pp2:
                arin = spool.tile([4, 4], F32)
                for oh in range(2):
                    ch2 = spool.tile([CIN, 2], F32, tag="ch2")
                    nc.vector.bn_aggr(
                        ch2[:], st2[:, oh].rearrange("p a b -> p (a b)")
                        .rearrange("p (n t) -> p n t", t=3))
                    me2 = spool.tile([CIN, 2], F32, tag="me2")
                    nc.vector.tensor_copy(me2[:, 0:1], ch2[:, 0:1])
                    t2 = spool.tile([CIN, 1], F32, tag="t2")
                    nc.vector.tensor_mul(t2[:], ch2[:, 0:1], ch2[:, 0:1])
                    nc.vector.tensor_add(me2[:, 1:2], ch2[:, 1:2], t2[:])
                    gp2 = pp2.tile([4, 2], F32)
                    nc.tensor.matmul(gp2[:], g2m[:], me2[:])
                    nc.scalar.mul(arin[:, oh * 2:(oh + 1) * 2], gp2[:],
                                  float(NLOC))
                arb_in = drampool.tile([4, 4], F32)
                arb_out = drampool.tile([4, 4], F32)
                nc.sync.dma_start(arb_in[:], arin[:])
                nc.gpsimd.collective_compute(
                    "AllReduce", ALU.add,
                    replica_groups=[[0, 1, 2, 3], [4, 5, 6, 7]],
                    ins=[arb_in.opt()], outs=[arb_out.opt()])
                gm2 = spool.tile([4, 4], F32)
                nc.sync.dma_start(gm2[:], arb_out[:])
                nc.scalar.mul(gm2[:], gm2[:], 1.0 / NGLOB)
                gr2 = spool.tile([4, 2, 2], F32)   # [g, oh, (mu, rstd)]
                for oh in range(2):
                    gv2 = spool.tile([4, 1], F32, tag="gv2")
                    nc.vector.tensor_mul(gv2[:], gm2[:, 2 * oh: 2 * oh + 1],
                                         gm2[:, 2 * oh: 2 * oh + 1])
                    nc.vector.tensor_sub(gv2[:], gm2[:, 2 * oh + 1: 2 * oh + 2],
                                         gv2[:])
                    nc.vector.tensor_scalar_add(gv2[:], gv2[:], EPS)
                    nc.vector.tensor_copy(gr2[:, oh, 0:1],
                                          gm2[:, 2 * oh: 2 * oh + 1])
                    nc.vector.reciprocal(gr2[:, oh, 1:2], gv2[:])
                    nc.scalar.sqrt(gr2[:, oh, 1:2], gr2[:, oh, 1:2])
                sc2 = spool.tile([CIN, 2, 2], F32)
                for oh in range(2):
                    bc2 = pp2.tile([CIN, 2], F32)
                    nc.tensor.matmul(bc2[:], g2tm[:], gr2[:, oh, :])
                    nc.vector.tensor_mul(sc2[:, oh, 0:1], c256[:, oh, 2:3],
                                         bc2[:, 1:2])
                    t3 = spool.tile([CIN, 1], F32, tag="t3")
                    nc.vector.tensor_mul(t3[:], bc2[:, 0:1], sc2[:, oh, 0:1])
                    nc.vector.tensor_sub(sc2[:, oh, 1:2], c256[:, oh, 3:4],
                                         t3[:])

                # ------------- P3: gn2 apply + gelu -> x_down quarter -------------
                with tc.tile_pool(name="p3", bufs=4) as p3:
                    for oh in range(2):
                        for h0 in range(0, HOUT, 13):
                            nh = min(13, HOUT - h0)
                            it = p3.tile([CIN, 13 * WOQ], F32, tag="p3i")
                            nc.sync.dma_start(
                                it[:, : nh * WOQ],
                                out_sc[oh, :, h0: h0 + nh, :]
                                .rearrange("p a b -> p (a b)"))
                            ot3 = p3.tile([CIN, 13 * WOQ], F32, tag="p3o")
                            nc.scalar.activation(
                                ot3[:, : nh * WOQ], it[:, : nh * WOQ],
                                AF.Gelu_apprx_tanh,
                                bias=sc2[:, oh, 1:2], scale=sc2[:, oh, 0:1])
                            nc.scalar.dma_start(
                                xdown_q[oh, :, h0: h0 + nh, :]
                                .rearrange("p a b -> p (a b)"),
                                ot3[:, : nh * WOQ])
    _legalize_waits(nc)
    return nc


def _prep_core_inputs(inputs, b, qw):
    x = np.asarray(inputs["x"], np.float32)
    rot = np.roll(x[b], -WQ * qw, axis=2)          # [128, 181, 360]
    w1 = np.asarray(inputs["w1"], np.float32)
    w2 = np.asarray(inputs["w2"], np.float32)
    weight = np.asarray(inputs["weight"], np.float32)
    psi_vals = np.asarray(inputs["psi_vals"], np.float32)

    m = {}
    m["xr_bf"] = rot.reshape(CIN, NPIX).astype(ml_dtypes.bfloat16)
    m["xr_f32q"] = np.ascontiguousarray(rot[:, :, :WQ])
    m["w1T"] = np.ascontiguousarray(w1.T).astype(ml_dtypes.bfloat16)
    w2t = np.ascontiguousarray(w2.T.reshape(2, CIN, CIN).transpose(1, 0, 2))
    m["w2T"] = w2t.astype(ml_dtypes.bfloat16)
    wtk = np.ascontiguousarray(weight.transpose(1, 2, 0))   # [c, k, o]
    m["wTk"] = wtk.astype(ml_dtypes.bfloat16)
    psibd = np.zeros((CIN, HT, 56), np.float32)
    for t in range(HT):
        for hl in range(8):
            hh = t * 8 + hl
            if hh >= HOUT:
                continue
            for n in range(NNZ):
                for k in range(K):
                    psibd[hl * 16 + n, t, hl * 7 + k] = psi_vals[k, hh, n]
    m["psiBD"] = psibd.astype(ml_dtypes.bfloat16)
    c128 = np.stack([np.asarray(inputs["gn1_gamma"], np.float32),
                     np.asarray(inputs["gn1_beta"], np.float32),
                     np.asarray(inputs["b2"], np.float32)], axis=1)
    m["c128"] = np.ascontiguousarray(c128)
    b1 = np.asarray(inputs["b1"], np.float32).reshape(2, CIN)
    bias = np.asarray(inputs["bias"], np.float32).reshape(2, CIN)
    gamma2 = np.asarray(inputs["gamma2"], np.float32).reshape(2, CIN)
    beta2 = np.asarray(inputs["beta2"], np.float32).reshape(2, CIN)
    c256 = np.stack([b1, bias, gamma2, beta2], axis=2).transpose(1, 0, 2)
    m["c256"] = np.ascontiguousarray(c256)
    g1 = np.zeros((CIN, 8), np.float32)
    for c in range(CIN):
        g1[c, c // 16] = 1.0
    m["g1"] = g1
    m["g1t"] = np.ascontiguousarray(g1.T)
    g2 = np.zeros((CIN, 4), np.float32)
    for c in range(CIN):
        g2[c, c // 32] = 1.0
    m["g2"] = g2
    m["g2t"] = np.ascontiguousarray(g2.T)
    return m


def kernel(**inputs):
    psi_hi = np.asarray(inputs["psi_hi"]).astype(np.int64)
    psi_wi = np.asarray(inputs["psi_wi"]).astype(np.int64)

    key = (psi_hi.tobytes(), psi_wi.tobytes())
    if key not in _CACHE:
        _CACHE.clear()
        _CACHE[key] = _build_program(psi_hi, psi_wi)
    nc = _CACHE[key]

    in_maps = []
    core_ids = list(range(8))
    for cid in core_ids:
        b, qw = cid // NW, cid % NW
        in_maps.append(_prep_core_inputs(inputs, b, qw))

    res = run_bass_kernel_spmd(nc, in_maps, core_ids)
    results = res.results

    skip = np.zeros((B, CIN, HIN, WIN), np.float32)
    x_down = np.zeros((B, COUT, HOUT, WOUT), np.float32)
    for cid in core_ids:
        b, qw = cid // NW, cid % NW
        r = results[cid]
        skip[b, :, :, WQ * qw: WQ * (qw + 1)] = r["skip_q"]
        xd = r["xdown_q"].reshape(COUT, HOUT, WOQ)
        x_down[b, :, :, WOQ * qw: WOQ * (qw + 1)] = xd
    return (skip, x_down)
